# revision 10
# baseline (speedup 1.0000x reference)
"""BertSelfAttention on 8 TRN2 NeuronCores.

B=4, S=2048, H=768, NH=12, HD=64. Sharding: core c <- (batch c//2,
head-group c%2 of 6 heads). No collectives; host shards/gathers.

Device kernel (per core, bf16 matmuls / fp32 PSUM):
  - qT/kT projections: weights stationary -> [2*64 dims, S] per head pair
  - v projection: hiddenT stationary -> natural [s, d] layout + ones column
  - scoresT = kT.T-major: scores computed transposed [k, q] so the exp'd
    probs feed the ctx matmul directly (contraction k on partitions);
    K=64 contraction row-packed 2 heads per pass via tile_position
  - softmax without max-subtraction (scores ~ N(0,1)); exp on ACT with
    scale=1/8 and mask as per-partition bias; denominator = ones column
  - output per head: [65, S] = unnormalized ctxT + denominator row;
    host divides + transposes during gather
  - kc-PAIR clustering: both score pairs (64-row-tiled PE mode) emit back
    to back, then all 128-row work (ctx + woven proj) for the pair: the
    64<->128-row retile drain (~105ns each way) is paid once per 2 kcs
    instead of per kc; steady-state MMs then run at pure stream time
    (~216ns for N=512 bf16) with LDWEIGHTS fully hidden
  - load phase: dma_start issue cost scales with partition rows (~0.7us
    per 128-row transfer), so the critical first bytes are only the
    pair-0 column slices of wk/wq + ht q0, round-robined over the three
    DMA queues; the tail splits the last unit's evacs into column halves
    across queues so the post-last-matmul serial tail is ~1 half-DMA
"""

import sys

sys.path.insert(0, "/opt/trn_rl_repo")

import numpy as np
import ml_dtypes

import concourse.bacc as bacc
import concourse.mybir as mybir
import concourse.tile as tile
from concourse.bass_utils import run_bass_kernel_spmd

B, S, H, NH, HD = 4, 2048, 768, 12, 64
N_CORES = 8
HEADS_PER_CORE = NH // 2  # 6
N_PAIR = HEADS_PER_CORE // 2  # 3
CCH = H // 128  # 6 contraction chunks
QC = 512  # q chunk width (moving dim)
QCH = S // QC  # 4
KCH = S // 128  # 16 k chunks
SCALE = 1.0 / float(np.sqrt(HD))

BF16 = mybir.dt.bfloat16
F32 = mybir.dt.float32
I16 = mybir.dt.int16

# DVE-offloaded exp: ex = 2^(t) with t = s*SCALE*log2e + mask*log2e,
# approximated in bf16 bit space (Schraudolph): bits16 = floor(t*128 +
# (127-C)*128 + mask*128*log2e + 0.5). One tensor_scalar per kc group
# (mult+add, f32 PSUM -> int16 SBUF), consumed via bitcast as bf16.
# Softmax num/den share the approx error, so it largely cancels; host
# sim: 6/16 groups offloaded -> rel err ~9e-3 (gate 2e-2).
LOG2E = 1.4426950408889634
A16 = 128.0 * LOG2E * SCALE
B16 = 128.0 * (127.0 - 0.0573) + 0.5
# even kcs on DVE: strict ACT/DVE alternation (incl. across unit
# boundaries) keeps both consumers ahead of the PE's score-pair cadence
# (all-core host sim @ C=0.0573: ~1.2e-2 < 2e-2 gate; odd-kc sets
# measured worse on device)
DVE_KC = (0, 2, 4, 6, 8, 10, 12, 14)

_NC_CACHE = {}


def _build_nc():
    nc = bacc.Bacc("TRN2", target_bir_lowering=False)

    ht_ext = nc.dram_tensor("ht", [H, S], BF16, kind="ExternalInput")
    wq_ext = nc.dram_tensor("wq", [H, HEADS_PER_CORE * HD], BF16, kind="ExternalInput")
    wk_ext = nc.dram_tensor("wk", [H, HEADS_PER_CORE * HD], BF16, kind="ExternalInput")
    wv_ext = nc.dram_tensor("wv", [H, HEADS_PER_CORE * HD], BF16, kind="ExternalInput")
    mask_ext = nc.dram_tensor("mask", [128, KCH], F32, kind="ExternalInput")
    out_ext = nc.dram_tensor(
        "out", [HEADS_PER_CORE, HD + 1, S], F32, kind="ExternalOutput"
    )

    with tile.TileContext(nc) as tc:
        with (
            tc.tile_pool(name="const", bufs=1) as const,
            tc.tile_pool(name="qk", bufs=1) as qk,
            tc.tile_pool(name="expp", bufs=4) as expp,
            tc.tile_pool(name="outp", bufs=3) as outp,
            tc.tile_pool(name="pj_ps", bufs=2, space="PSUM") as pj_ps,
            tc.tile_pool(name="sc_ps", bufs=2, space="PSUM") as sc_ps,
            # two tags (cx0/cx1) x bufs=1 -> 2 banks
            tc.tile_pool(name="cx_ps", bufs=1, space="PSUM") as cx_ps,
        ):
            # ---- loads, ordered for the first kT/qT chains' critical path.
            # dma_start issue cost scales with partition rows (~0.7us per
            # 128-row transfer on the issuing queue), so the critical path
            # is (a) queue-issue serialization and (b) the first transfers'
            # bytes. The first kt chain needs ONLY the pair-0 column slice
            # of wk (cols 0:128) plus ht q0 — load exactly that first,
            # striped over FOUR queues (scalar+vector are idle until the
            # exp stream starts ~20us in).
            mask_sb = const.tile([128, KCH], F32, tag="mask")
            m16 = const.tile([128, KCH], F32, tag="m16")
            w_sb = {
                name: const.tile(
                    [128, CCH, HEADS_PER_CORE * HD], BF16, tag=name, name=name
                )
                for name in ("wq", "wk", "wv")
            }
            ht_sb = const.tile([128, CCH, S], BF16, tag="ht")

            def _w(name, ext, cc, eng):
                eng.dma_start(
                    out=w_sb[name][:, cc, :],
                    in_=ext[cc * 128 : (cc + 1) * 128, :],
                )

            def _wp(name, ext, cc, p, eng):
                eng.dma_start(
                    out=w_sb[name][:, cc, p * 128 : (p + 1) * 128],
                    in_=ext[cc * 128 : (cc + 1) * 128, p * 128 : (p + 1) * 128],
                )

            def _hq(cc, q, eng):
                eng.dma_start(
                    out=ht_sb[:, cc, q * QC : (q + 1) * QC],
                    in_=ht_ext[cc * 128 : (cc + 1) * 128, q * QC : (q + 1) * QC],
                )

            # wave 1 (critical): wk pair-0 slices + ht q0, round-robin on
            # the three DMA-capable queues (sync/gpsimd/scalar)
            q3 = [nc.sync, nc.gpsimd, nc.scalar]
            for cc in range(CCH):
                _wp("wk", wk_ext, cc, 0, q3[cc % 3])
                _hq(cc, 0, q3[(cc + 2) % 3])
            # mask (exp bias + ACT warm) behind the critical wk slices
            nc.sync.dma_start(out=mask_sb[:], in_=mask_ext[:])
            # per-partition bias for the DVE exp groups: mask*128*log2e + B16
            nc.vector.tensor_scalar(
                out=m16[:],
                in0=mask_sb[:],
                scalar1=128.0 * LOG2E,
                scalar2=B16,
                op0=mybir.AluOpType.mult,
                op1=mybir.AluOpType.add,
            )
            # wave 2: wq pair-0 (qt chain 0 follows the kt chain)
            for cc in range(CCH):
                _wp("wq", wq_ext, cc, 0, q3[(cc + 1) % 3])
            # dummy 1-elem exp after scalar's critical-DMA share: pulls the
            # ~2.7us ACT table load into the load phase, before the first
            # real score activation
            warm = const.tile([128, 1], F32, tag="warm")
            nc.scalar.activation(
                out=warm[:],
                in_=mask_sb[:, 0:1],
                func=mybir.ActivationFunctionType.Exp,
            )
            # non-critical remainder on sync/gpsimd only (scalar/vector must
            # stay clear for the exp stream): ht q1 (chains sch1), wv (v
            # proj, woven from unit (0,1)), then the remaining wk/wq pair
            # slices (pair-1/2 chains start ~45us in) and ht q2/q3
            for cc in range(CCH):
                _hq(cc, 1, nc.sync if cc % 2 == 0 else nc.gpsimd)
            for cc in range(CCH):
                _w("wv", wv_ext, cc, nc.gpsimd)
                _wp("wk", wk_ext, cc, 1, nc.sync)
            for cc in range(CCH):
                _wp("wq", wq_ext, cc, 1, nc.sync)
                _wp("wk", wk_ext, cc, 2, nc.gpsimd)
            for cc in range(CCH):
                _wp("wq", wq_ext, cc, 2, nc.gpsimd)
                _hq(cc, 2, nc.sync)
            for cc in range(CCH):
                _hq(cc, 3, nc.sync if cc % 2 == 0 else nc.gpsimd)

            # v_aug[p, kc, h, 0:64] = v, [..., 64] = 1.0 (denominator column)
            vaug = const.tile([128, KCH, HEADS_PER_CORE, HD + 1], BF16, tag="vaug")
            nc.vector.memset(vaug[:, :, :, HD : HD + 1], 1.0)

            qt_sb = [
                qk.tile([128, S], BF16, name=f"qt{p}", tag=f"qt{p}")
                for p in range(N_PAIR)
            ]
            kt_sb = [
                qk.tile([128, S], BF16, name=f"kt{p}", tag=f"kt{p}")
                for p in range(N_PAIR)
            ]

            def chain_gen(p, name, sch):
                # lazy emission of one qT/kT projection chain: one
                # instruction per next() so it can be woven between score
                # pairs (the PE queue is strict FIFO)
                w = w_sb[name]
                dst = (qt_sb if name == "wq" else kt_sb)[p]
                ps = pj_ps.tile([128, QC], F32, tag="pj", name=f"pj_{name}{p}_{sch}")
                for cc in range(CCH):
                    nc.tensor.matmul(
                        ps[:],
                        w[:, cc, p * 128 : (p + 1) * 128],
                        ht_sb[:, cc, sch * QC : (sch + 1) * QC],
                        start=(cc == 0),
                        stop=(cc == CCH - 1),
                    )
                    yield
                nc.vector.tensor_copy(dst[:, sch * QC : (sch + 1) * QC], ps[:])
                yield

            def v_gen(kb):
                # one k-block of the v projection (all 6 heads)
                wv = w_sb["wv"]
                ps = pj_ps.tile(
                    [128, HEADS_PER_CORE * HD], F32, tag="pj", name=f"pv_{kb}"
                )
                for cc in range(CCH):
                    nc.tensor.matmul(
                        ps[:],
                        ht_sb[:, cc, kb * 128 : (kb + 1) * 128],
                        wv[:, cc, :],
                        start=(cc == 0),
                        stop=(cc == CCH - 1),
                    )
                    yield
                nc.vector.tensor_copy(
                    vaug[:, kb, :, 0:HD],
                    ps[:].rearrange("p (h d) -> p h d", h=HEADS_PER_CORE),
                )
                yield

            def weave(gens, budget):
                n = 0
                while gens and n < budget:
                    try:
                        next(gens[0])
                        n += 1
                    except StopIteration:
                        gens.pop(0)

            def drain(gens):
                while gens:
                    try:
                        next(gens[0])
                    except StopIteration:
                        gens.pop(0)

            def alloc_ex(p, j):
                return expp.tile([128, KCH, 2, QC], BF16, tag="ex", name=f"ex{p}_{j}")

            def ex_sl(ex, kc):
                return ex[:, kc]

            def scores_exp_part(p, j, ex, kcs):
                for kc in kcs:
                    sc = sc_ps.tile([128, 2, QC], F32, tag="sc", name=f"sc{p}_{j}_{kc}")
                    for h01 in range(2):
                        lo, hi = h01 * 64, h01 * 64 + 64
                        nc.tensor.matmul(
                            sc[:, h01, :],
                            kt_sb[p][lo:hi, kc * 128 : (kc + 1) * 128],
                            qt_sb[p][lo:hi, j * QC : (j + 1) * QC],
                            start=True,
                            stop=True,
                        )
                    if kc in DVE_KC:
                        nc.vector.tensor_scalar(
                            out=ex_sl(ex, kc).bitcast(I16),
                            in0=sc[:],
                            scalar1=A16,
                            scalar2=m16[:, kc : kc + 1],
                            op0=mybir.AluOpType.mult,
                            op1=mybir.AluOpType.add,
                        )
                    else:
                        nc.scalar.activation(
                            out=ex_sl(ex, kc),
                            in_=sc[:],
                            func=mybir.ActivationFunctionType.Exp,
                            scale=SCALE,
                            bias=mask_sb[:, kc : kc + 1],
                        )

            def scores_exp(p, j):
                ex = alloc_ex(p, j)
                scores_exp_part(p, j, ex, range(KCH))
                return ex

            def ctx_evac(cp, cj, cx0, cx1, on_vector=False):
                if on_vector:
                    # final unit: the copy+DMA is the serial tail after the
                    # last matmul — split each head's evac into column
                    # halves with the copies split across vector+scalar and
                    # the four DMAs on four different queues so the tail is
                    # ~one half-transfer long instead of two full ones
                    engs = ((nc.sync, nc.gpsimd), (nc.scalar, nc.sync))
                    for h01, cx in ((0, cx0), (1, cx1)):
                        o_sb = outp.tile(
                            [HD + 1, QC], F32, tag="o", name=f"o{cp}_{cj}_{h01}"
                        )
                        for half in range(2):
                            sl = slice(half * (QC // 2), (half + 1) * (QC // 2))
                            if h01 == 0:
                                nc.vector.tensor_copy(o_sb[:, sl], cx[:, sl])
                            else:
                                nc.scalar.copy(o_sb[:, sl], cx[:, sl])
                            engs[h01][half].dma_start(
                                out=out_ext[
                                    2 * cp + h01,
                                    :,
                                    cj * QC + half * (QC // 2) : cj * QC
                                    + (half + 1) * (QC // 2),
                                ],
                                in_=o_sb[:, sl],
                            )
                    return
                # steady state: DMAs ride sync+gpsimd; scalar engine (has
                # slack) evacuates ctx psum so the vector queue stays clear
                # for exp groups + proj casts
                for h01, cx, eng in ((0, cx0, nc.sync), (1, cx1, nc.gpsimd)):
                    o_sb = outp.tile(
                        [HD + 1, QC], F32, tag="o", name=f"o{cp}_{cj}_{h01}"
                    )
                    nc.scalar.copy(o_sb[:], cx[:])
                    eng.dma_start(
                        out=out_ext[2 * cp + h01, :, cj * QC : (cj + 1) * QC],
                        in_=o_sb[:],
                    )

            def fused_unit(p, j, cp, cj, cex, gens=(), budget=4, ctx2=None,
                           ctx_self=None):
                # scores+exp of unit (p, j) interleaved per-kc with the ctx
                # accumulation of unit (cp, cj) and woven projection chain
                # steps: the PE engine queue is strict FIFO, so emission
                # order must match the steady-state demand (one score pair
                # per ACT period, ctx + proj fill the rest) or ACT starves.
                gens = list(gens)
                ex = alloc_ex(p, j)
                cx0 = cx1 = None
                if cp is not None:
                    cx0 = cx_ps.tile([HD + 1, QC], F32, tag="cx0", name=f"cx0_{cp}_{cj}")
                    cx1 = cx_ps.tile([HD + 1, QC], F32, tag="cx1", name=f"cx1_{cp}_{cj}")
                dx0 = dx1 = None
                if ctx2 is not None:
                    dp, dj, _ = ctx2
                    dx0 = pj_ps.tile([HD + 1, QC], F32, tag="pj", name=f"cx0_{dp}_{dj}")
                    dx1 = pj_ps.tile([HD + 1, QC], F32, tag="pj", name=f"cx1_{dp}_{dj}")
                if ctx_self:
                    # this unit's own ctx rides the pj banks, consuming its
                    # own ex with a 2-kc lag (the exp of kc is ready by the
                    # time the PE reaches kc+2) — kills the serial PE tail
                    dx0 = pj_ps.tile([HD + 1, QC], F32, tag="pj", name=f"cxs0_{p}_{j}")
                    dx1 = pj_ps.tile([HD + 1, QC], F32, tag="pj", name=f"cxs1_{p}_{j}")

                def ctx_self_mm(kcc):
                    for h01, cx in ((0, dx0), (1, dx1)):
                        nc.tensor.matmul(
                            cx[:],
                            vaug[:, kcc, 2 * p + h01, :],
                            ex_sl(ex, kcc)[:, h01, :],
                            start=(kcc == 0),
                            stop=(kcc == KCH - 1),
                        )

                # kc-PAIR clustering: both score pairs (64-row-tiled mode)
                # back to back, then all 128-row work (proj weave + ctx) for
                # the pair — 2 PE mode-switch drains per 2 kcs instead of 4
                for kcp in range(KCH // 2):
                    kc0, kc1 = 2 * kcp, 2 * kcp + 1
                    scores_exp_part(p, j, ex, [kc0])
                    scores_exp_part(p, j, ex, [kc1])
                    weave(gens, budget)
                    for kc in (kc0, kc1):
                        if cp is not None:
                            for h01, cx in ((0, cx0), (1, cx1)):
                                nc.tensor.matmul(
                                    cx[:],
                                    vaug[:, kc, 2 * cp + h01, :],
                                    ex_sl(cex, kc)[:, h01, :],
                                    start=(kc == 0),
                                    stop=(kc == KCH - 1),
                                )
                        if ctx2 is not None:
                            dp, dj, dex = ctx2
                            for h01, cx in ((0, dx0), (1, dx1)):
                                nc.tensor.matmul(
                                    cx[:],
                                    vaug[:, kc, 2 * dp + h01, :],
                                    ex_sl(dex, kc)[:, h01, :],
                                    start=(kc == 0),
                                    stop=(kc == KCH - 1),
                                )
                    if ctx_self and kcp >= 1:
                        ctx_self_mm(kc0 - 2)
                        ctx_self_mm(kc1 - 2)
                drain(gens)
                if ctx_self:
                    ctx_self_mm(KCH - 2)
                    ctx_self_mm(KCH - 1)
                if cp is not None:
                    ctx_evac(cp, cj, cx0, cx1)
                if ctx2 is not None:
                    dp, dj, _ = ctx2
                    ctx_evac(dp, dj, dx0, dx1)
                if ctx_self:
                    ctx_evac(p, j, dx0, dx1, on_vector=True)
                return ex

            # Pair 0 is special-ordered so ACT starts as early as possible:
            # scores/exp need only qT/kT; v-projection matmuls fill PE gaps
            # while ACT chews exps; ctx comes after proj_v (vaug dependency).
            # Next pair's projection chains are spread through the current
            # pair's attention units so the scheduler can hide them in the
            # ACT-gated gaps instead of paying for them at pair boundaries.
            # Startup: first kT/qT chains run as blocks (critical path),
            # then everything — remaining chains, the v projection, next
            # pair's chains — is woven between score pairs at kc grain.
            drain([chain_gen(0, "wk", 0)])
            drain([chain_gen(0, "wq", 0)])
            ex00 = alloc_ex(0, 0)
            g = [chain_gen(0, "wk", 1), chain_gen(0, "wk", 2),
                 chain_gen(0, "wk", 3), chain_gen(0, "wq", 1)]
            for kcp in range(KCH // 2):
                scores_exp_part(0, 0, ex00, [2 * kcp])
                scores_exp_part(0, 0, ex00, [2 * kcp + 1])
                weave(g, 4)
            drain(g)
            ex01 = alloc_ex(0, 1)
            g = [chain_gen(0, "wq", 2)] + [v_gen(kb) for kb in range(KCH)]
            for kcp in range(KCH // 2):
                scores_exp_part(0, 1, ex01, [2 * kcp])
                scores_exp_part(0, 1, ex01, [2 * kcp + 1])
                weave(g, 6)
            drain(g)  # ctx(0,0) in the next unit needs all of v
            ex02 = fused_unit(0, 2, 0, 0, ex00, [chain_gen(0, "wq", 3)])
            ex03 = fused_unit(
                0, 3, 0, 1, ex01,
                [chain_gen(1, "wk", 0), chain_gen(1, "wk", 1), chain_gen(1, "wq", 0)],
            )
            ex10 = fused_unit(
                1, 0, 0, 2, ex02,
                [chain_gen(1, "wk", 2), chain_gen(1, "wk", 3), chain_gen(1, "wq", 1)],
            )
            ex11 = fused_unit(
                1, 1, 0, 3, ex03, [chain_gen(1, "wq", 2), chain_gen(2, "wk", 0)]
            )
            ex12 = fused_unit(
                1, 2, 1, 0, ex10, [chain_gen(1, "wq", 3), chain_gen(2, "wk", 1)]
            )
            ex13 = fused_unit(
                1, 3, 1, 1, ex11, [chain_gen(2, "wk", 2), chain_gen(2, "wq", 0)]
            )
            ex20 = fused_unit(
                2, 0, 1, 2, ex12, [chain_gen(2, "wk", 3), chain_gen(2, "wq", 1)]
            )
            ex21 = fused_unit(
                2, 1, 1, 3, ex13, [chain_gen(2, "wq", 2), chain_gen(2, "wq", 3)]
            )
            # tail shrink: the last two units each carry TWO ctx units —
            # one on the cx banks, one on the now-idle proj banks; the very
            # last unit self-consumes its own ex at a 2-kc lag so only two
            # ctx chunks trail the final scores
            ex22 = fused_unit(2, 2, 2, 0, ex20, ctx2=(2, 1, ex21))
            ex23 = fused_unit(2, 3, 2, 2, ex22, ctx_self=True)

    nc.compile()
    return nc


def _get_nc():
    if "nc" not in _NC_CACHE:
        _NC_CACHE["nc"] = _build_nc()
    return _NC_CACHE["nc"]


def _make_in_maps(hidden, mask, Wq, Wk, Wv):
    bf16 = ml_dtypes.bfloat16
    in_maps = []
    for c in range(N_CORES):
        b, hg = c // 2, c % 2
        cols = slice(hg * HEADS_PER_CORE * HD, (hg + 1) * HEADS_PER_CORE * HD)
        mc = np.ascontiguousarray(
            mask[b, 0, 0].astype(np.float32).reshape(KCH, 128).T
        )
        in_maps.append(
            {
                "ht": np.ascontiguousarray(hidden[b].T).astype(bf16),
                "wq": np.ascontiguousarray(Wq[:, cols]).astype(bf16),
                "wk": np.ascontiguousarray(Wk[:, cols]).astype(bf16),
                "wv": np.ascontiguousarray(Wv[:, cols]).astype(bf16),
                "mask": mc,
            }
        )
    return in_maps


def _gather(results):
    out = np.empty((B, S, H), dtype=np.float32)
    for c in range(N_CORES):
        b, hg = c // 2, c % 2
        r = results[c]["out"]  # [6, 65, S]
        num = r[:, :HD, :]  # [6, 64, S]
        den = r[:, HD : HD + 1, :]  # [6, 1, S]
        ctx = np.transpose(num / den, (2, 0, 1)).reshape(S, HEADS_PER_CORE * HD)
        out[b, :, hg * HEADS_PER_CORE * HD : (hg + 1) * HEADS_PER_CORE * HD] = ctx
    return out


def _run_device(hidden, mask, Wq, Wk, Wv, trace=False):
    nc = _get_nc()
    in_maps = _make_in_maps(hidden, mask, Wq, Wk, Wv)
    res = run_bass_kernel_spmd(nc, in_maps, core_ids=list(range(N_CORES)), trace=trace)
    return _gather(res.results), res


def _numpy_fallback(hidden_states, attention_mask, Wq, bq, Wk, bk, Wv, bv):
    def split_heads(x):
        return x.reshape(B, S, NH, HD).transpose(0, 2, 1, 3)

    q = split_heads(hidden_states @ Wq + bq)
    k = split_heads(hidden_states @ Wk + bk)
    v = split_heads(hidden_states @ Wv + bv)
    scores = np.einsum("bhqd,bhkd->bhqk", q, k) / np.sqrt(HD) + attention_mask
    scores -= scores.max(axis=-1, keepdims=True)
    e = np.exp(scores)
    probs = e / e.sum(axis=-1, keepdims=True)
    ctx = np.einsum("bhqk,bhkd->bhqd", probs, v)
    return ctx.transpose(0, 2, 1, 3).reshape(B, S, H).astype(np.float32)


def kernel(hidden_states, attention_mask, Wq, bq, Wk, bk, Wv, bv):
    hidden = np.asarray(hidden_states, dtype=np.float32)
    mask = np.asarray(attention_mask, dtype=np.float32)
    Wq = np.asarray(Wq, dtype=np.float32)
    Wk = np.asarray(Wk, dtype=np.float32)
    Wv = np.asarray(Wv, dtype=np.float32)
    bq, bk, bv = (np.asarray(x, dtype=np.float32) for x in (bq, bk, bv))
    if np.any(bq) or np.any(bk) or np.any(bv):
        # projection biases are zero for this problem; keep a correct
        # fallback rather than a dead device path
        return _numpy_fallback(hidden, mask, Wq, bq, Wk, bk, Wv, bv)
    out, _ = _run_device(hidden, mask, Wq, Wk, Wv)
    return out



# revision 12
# speedup vs baseline: 1.0160x; 1.0160x over previous
"""BertSelfAttention on 8 TRN2 NeuronCores.

B=4, S=2048, H=768, NH=12, HD=64. Sharding: core c <- (batch c//2,
head-group c%2 of 6 heads). No collectives; host shards/gathers.

Device kernel (per core, bf16 matmuls / fp32 PSUM):
  - qT/kT projections: weights stationary -> [2*64 dims, S] per head pair
  - v projection: hiddenT stationary -> natural [s, d] layout + ones column
  - scoresT = kT.T-major: scores computed transposed [k, q] so the exp'd
    probs feed the ctx matmul directly (contraction k on partitions);
    K=64 contraction row-packed 2 heads per pass via tile_position
  - softmax without max-subtraction (scores ~ N(0,1)); exp on ACT with
    scale=1/8 and mask as per-partition bias; denominator = ones column
  - output per head: [65, S] = unnormalized ctxT + denominator row;
    host divides + transposes during gather
  - kc-PAIR clustering: both score pairs (64-row-tiled PE mode) emit back
    to back, then all 128-row work (ctx + woven proj) for the pair: the
    64<->128-row retile drain (~105ns each way) is paid once per 2 kcs
    instead of per kc; steady-state MMs then run at pure stream time
    (~216ns for N=512 bf16) with LDWEIGHTS fully hidden
  - tail: the last unit's evacs split into column halves with the four
    DMAs spread over the three DMA queues so the post-last-matmul serial
    tail is ~1 half-transfer (finer load-phase striping of wk/wq was
    tried and REGRESSED ~6us — extra dma_start issues delay the queues)
"""

import sys

sys.path.insert(0, "/opt/trn_rl_repo")

import numpy as np
import ml_dtypes

import concourse.bacc as bacc
import concourse.mybir as mybir
import concourse.tile as tile
from concourse.bass_utils import run_bass_kernel_spmd

B, S, H, NH, HD = 4, 2048, 768, 12, 64
N_CORES = 8
HEADS_PER_CORE = NH // 2  # 6
N_PAIR = HEADS_PER_CORE // 2  # 3
CCH = H // 128  # 6 contraction chunks
QC = 512  # q chunk width (moving dim)
QCH = S // QC  # 4
KCH = S // 128  # 16 k chunks
SCALE = 1.0 / float(np.sqrt(HD))

BF16 = mybir.dt.bfloat16
F32 = mybir.dt.float32
I16 = mybir.dt.int16

# DVE-offloaded exp: ex = 2^(t) with t = s*SCALE*log2e + mask*log2e,
# approximated in bf16 bit space (Schraudolph): bits16 = floor(t*128 +
# (127-C)*128 + mask*128*log2e + 0.5). One tensor_scalar per kc group
# (mult+add, f32 PSUM -> int16 SBUF), consumed via bitcast as bf16.
# Softmax num/den share the approx error, so it largely cancels; host
# sim: 6/16 groups offloaded -> rel err ~9e-3 (gate 2e-2).
LOG2E = 1.4426950408889634
A16 = 128.0 * LOG2E * SCALE
B16 = 128.0 * (127.0 - 0.0573) + 0.5
# even kcs on DVE: strict ACT/DVE alternation (incl. across unit
# boundaries) keeps both consumers ahead of the PE's score-pair cadence
# (all-core host sim @ C=0.0573: ~1.2e-2 < 2e-2 gate; odd-kc sets
# measured worse on device)
DVE_KC = (0, 2, 4, 6, 8, 10, 12, 14)

_NC_CACHE = {}


def _build_nc():
    nc = bacc.Bacc("TRN2", target_bir_lowering=False)

    ht_ext = nc.dram_tensor("ht", [H, S], BF16, kind="ExternalInput")
    wq_ext = nc.dram_tensor("wq", [H, HEADS_PER_CORE * HD], BF16, kind="ExternalInput")
    wk_ext = nc.dram_tensor("wk", [H, HEADS_PER_CORE * HD], BF16, kind="ExternalInput")
    wv_ext = nc.dram_tensor("wv", [H, HEADS_PER_CORE * HD], BF16, kind="ExternalInput")
    mask_ext = nc.dram_tensor("mask", [128, KCH], F32, kind="ExternalInput")
    out_ext = nc.dram_tensor(
        "out", [HEADS_PER_CORE, HD + 1, S], F32, kind="ExternalOutput"
    )

    with tile.TileContext(nc) as tc:
        with (
            tc.tile_pool(name="const", bufs=1) as const,
            tc.tile_pool(name="qk", bufs=1) as qk,
            tc.tile_pool(name="expp", bufs=4) as expp,
            tc.tile_pool(name="outp", bufs=3) as outp,
            tc.tile_pool(name="pj_ps", bufs=2, space="PSUM") as pj_ps,
            tc.tile_pool(name="sc_ps", bufs=2, space="PSUM") as sc_ps,
            # two tags (cx0/cx1) x bufs=1 -> 2 banks
            tc.tile_pool(name="cx_ps", bufs=1, space="PSUM") as cx_ps,
        ):
            # ---- loads, ordered for the first kT/qT chains' critical path.
            # dma_start issue cost scales with partition rows (~0.7us per
            # 128-row transfer on the issuing queue), so the critical path
            # is (a) queue-issue serialization and (b) the first transfers'
            # bytes. The first kt chain needs ONLY the pair-0 column slice
            # of wk (cols 0:128) plus ht q0 — load exactly that first,
            # striped over FOUR queues (scalar+vector are idle until the
            # exp stream starts ~20us in).
            mask_sb = const.tile([128, KCH], F32, tag="mask")
            m16 = const.tile([128, KCH], F32, tag="m16")
            w_sb = {
                name: const.tile(
                    [128, CCH, HEADS_PER_CORE * HD], BF16, tag=name, name=name
                )
                for name in ("wq", "wk", "wv")
            }
            ht_sb = const.tile([128, CCH, S], BF16, tag="ht")

            def _w(name, ext, cc, eng):
                eng.dma_start(
                    out=w_sb[name][:, cc, :],
                    in_=ext[cc * 128 : (cc + 1) * 128, :],
                )

            def _wp(name, ext, cc, p, eng):
                eng.dma_start(
                    out=w_sb[name][:, cc, p * 128 : (p + 1) * 128],
                    in_=ext[cc * 128 : (cc + 1) * 128, p * 128 : (p + 1) * 128],
                )

            def _hq(cc, q, eng):
                eng.dma_start(
                    out=ht_sb[:, cc, q * QC : (q + 1) * QC],
                    in_=ht_ext[cc * 128 : (cc + 1) * 128, q * QC : (q + 1) * QC],
                )

            # mask first on sync (exp bias + ACT warm dependency)
            nc.sync.dma_start(out=mask_sb[:], in_=mask_ext[:])
            # per-partition bias for the DVE exp groups: mask*128*log2e + B16
            nc.vector.tensor_scalar(
                out=m16[:],
                in0=mask_sb[:],
                scalar1=128.0 * LOG2E,
                scalar2=B16,
                op0=mybir.AluOpType.mult,
                op1=mybir.AluOpType.add,
            )
            # first chains (kt/qt pair0 sch0) need only wk+wq and ht q-chunk
            # 0: load that critical set first, striped over THREE queues
            # (scalar is idle during the load phase; 3 queues ~ HBM cap),
            # cc-interleaved so the woven chain matmuls start as chunks land
            q3 = [nc.sync, nc.gpsimd, nc.scalar]
            for cc in range(CCH):
                _w("wk", wk_ext, cc, q3[cc % 3])
                _w("wq", wq_ext, cc, q3[(cc + 1) % 3])
                _hq(cc, 0, q3[(cc + 2) % 3])
            # dummy 1-elem exp after scalar's critical-DMA share: pulls the
            # ~2.7us ACT table load into the load phase, before the first
            # real score activation
            warm = const.tile([128, 1], F32, tag="warm")
            nc.scalar.activation(
                out=warm[:],
                in_=mask_sb[:, 0:1],
                func=mybir.ActivationFunctionType.Exp,
            )
            # non-critical remainder on sync/gpsimd only (scalar must stay
            # clear for the exp stream): ht q1 (chains sch1) and wv (v
            # proj, woven from unit (0,1)) trickle in PARALLEL on separate
            # queues so neither consumer stalls, then ht q2/q3
            for cc in range(CCH):
                _hq(cc, 1, nc.sync)
                _w("wv", wv_ext, cc, nc.gpsimd)
            for q in range(2, QCH):
                for cc in range(CCH):
                    _hq(cc, q, nc.sync if (cc + q) % 2 == 0 else nc.gpsimd)

            # v_aug[p, kc, h, 0:64] = v, [..., 64] = 1.0 (denominator column)
            vaug = const.tile([128, KCH, HEADS_PER_CORE, HD + 1], BF16, tag="vaug")
            nc.vector.memset(vaug[:, :, :, HD : HD + 1], 1.0)

            qt_sb = [
                qk.tile([128, S], BF16, name=f"qt{p}", tag=f"qt{p}")
                for p in range(N_PAIR)
            ]
            kt_sb = [
                qk.tile([128, S], BF16, name=f"kt{p}", tag=f"kt{p}")
                for p in range(N_PAIR)
            ]

            def chain_gen(p, name, sch):
                # lazy emission of one qT/kT projection chain: one
                # instruction per next() so it can be woven between score
                # pairs (the PE queue is strict FIFO)
                w = w_sb[name]
                dst = (qt_sb if name == "wq" else kt_sb)[p]
                ps = pj_ps.tile([128, QC], F32, tag="pj", name=f"pj_{name}{p}_{sch}")
                for cc in range(CCH):
                    nc.tensor.matmul(
                        ps[:],
                        w[:, cc, p * 128 : (p + 1) * 128],
                        ht_sb[:, cc, sch * QC : (sch + 1) * QC],
                        start=(cc == 0),
                        stop=(cc == CCH - 1),
                    )
                    yield
                nc.vector.tensor_copy(dst[:, sch * QC : (sch + 1) * QC], ps[:])
                yield

            def v_gen(kb):
                # one k-block of the v projection (all 6 heads)
                wv = w_sb["wv"]
                ps = pj_ps.tile(
                    [128, HEADS_PER_CORE * HD], F32, tag="pj", name=f"pv_{kb}"
                )
                for cc in range(CCH):
                    nc.tensor.matmul(
                        ps[:],
                        ht_sb[:, cc, kb * 128 : (kb + 1) * 128],
                        wv[:, cc, :],
                        start=(cc == 0),
                        stop=(cc == CCH - 1),
                    )
                    yield
                nc.vector.tensor_copy(
                    vaug[:, kb, :, 0:HD],
                    ps[:].rearrange("p (h d) -> p h d", h=HEADS_PER_CORE),
                )
                yield

            def weave(gens, budget):
                n = 0
                while gens and n < budget:
                    try:
                        next(gens[0])
                        n += 1
                    except StopIteration:
                        gens.pop(0)

            def drain(gens):
                while gens:
                    try:
                        next(gens[0])
                    except StopIteration:
                        gens.pop(0)

            def alloc_ex(p, j):
                return expp.tile([128, KCH, 2, QC], BF16, tag="ex", name=f"ex{p}_{j}")

            def ex_sl(ex, kc):
                return ex[:, kc]

            def scores_exp_part(p, j, ex, kcs):
                for kc in kcs:
                    sc = sc_ps.tile([128, 2, QC], F32, tag="sc", name=f"sc{p}_{j}_{kc}")
                    for h01 in range(2):
                        lo, hi = h01 * 64, h01 * 64 + 64
                        nc.tensor.matmul(
                            sc[:, h01, :],
                            kt_sb[p][lo:hi, kc * 128 : (kc + 1) * 128],
                            qt_sb[p][lo:hi, j * QC : (j + 1) * QC],
                            start=True,
                            stop=True,
                        )
                    if kc in DVE_KC:
                        nc.vector.tensor_scalar(
                            out=ex_sl(ex, kc).bitcast(I16),
                            in0=sc[:],
                            scalar1=A16,
                            scalar2=m16[:, kc : kc + 1],
                            op0=mybir.AluOpType.mult,
                            op1=mybir.AluOpType.add,
                        )
                    else:
                        nc.scalar.activation(
                            out=ex_sl(ex, kc),
                            in_=sc[:],
                            func=mybir.ActivationFunctionType.Exp,
                            scale=SCALE,
                            bias=mask_sb[:, kc : kc + 1],
                        )

            def scores_exp(p, j):
                ex = alloc_ex(p, j)
                scores_exp_part(p, j, ex, range(KCH))
                return ex

            def ctx_evac(cp, cj, cx0, cx1, on_vector=False):
                if on_vector:
                    # final unit: the copy+DMA is the serial tail after the
                    # last matmul — split each head's evac into column
                    # halves with the copies split across vector+scalar and
                    # the four DMAs on four different queues so the tail is
                    # ~one half-transfer long instead of two full ones
                    engs = ((nc.sync, nc.gpsimd), (nc.scalar, nc.sync))
                    for h01, cx in ((0, cx0), (1, cx1)):
                        o_sb = outp.tile(
                            [HD + 1, QC], F32, tag="o", name=f"o{cp}_{cj}_{h01}"
                        )
                        for half in range(2):
                            sl = slice(half * (QC // 2), (half + 1) * (QC // 2))
                            if h01 == 0:
                                nc.vector.tensor_copy(o_sb[:, sl], cx[:, sl])
                            else:
                                nc.scalar.copy(o_sb[:, sl], cx[:, sl])
                            engs[h01][half].dma_start(
                                out=out_ext[
                                    2 * cp + h01,
                                    :,
                                    cj * QC + half * (QC // 2) : cj * QC
                                    + (half + 1) * (QC // 2),
                                ],
                                in_=o_sb[:, sl],
                            )
                    return
                # steady state: DMAs ride sync+gpsimd; scalar engine (has
                # slack) evacuates ctx psum so the vector queue stays clear
                # for exp groups + proj casts
                for h01, cx, eng in ((0, cx0, nc.sync), (1, cx1, nc.gpsimd)):
                    o_sb = outp.tile(
                        [HD + 1, QC], F32, tag="o", name=f"o{cp}_{cj}_{h01}"
                    )
                    nc.scalar.copy(o_sb[:], cx[:])
                    eng.dma_start(
                        out=out_ext[2 * cp + h01, :, cj * QC : (cj + 1) * QC],
                        in_=o_sb[:],
                    )

            def fused_unit(p, j, cp, cj, cex, gens=(), budget=4, ctx2=None,
                           ctx_self=None):
                # scores+exp of unit (p, j) interleaved per-kc with the ctx
                # accumulation of unit (cp, cj) and woven projection chain
                # steps: the PE engine queue is strict FIFO, so emission
                # order must match the steady-state demand (one score pair
                # per ACT period, ctx + proj fill the rest) or ACT starves.
                gens = list(gens)
                ex = alloc_ex(p, j)
                cx0 = cx1 = None
                if cp is not None:
                    cx0 = cx_ps.tile([HD + 1, QC], F32, tag="cx0", name=f"cx0_{cp}_{cj}")
                    cx1 = cx_ps.tile([HD + 1, QC], F32, tag="cx1", name=f"cx1_{cp}_{cj}")
                dx0 = dx1 = None
                if ctx2 is not None:
                    dp, dj, _ = ctx2
                    dx0 = pj_ps.tile([HD + 1, QC], F32, tag="pj", name=f"cx0_{dp}_{dj}")
                    dx1 = pj_ps.tile([HD + 1, QC], F32, tag="pj", name=f"cx1_{dp}_{dj}")
                if ctx_self:
                    # this unit's own ctx rides the pj banks, consuming its
                    # own ex with a 2-kc lag (the exp of kc is ready by the
                    # time the PE reaches kc+2) — kills the serial PE tail
                    dx0 = pj_ps.tile([HD + 1, QC], F32, tag="pj", name=f"cxs0_{p}_{j}")
                    dx1 = pj_ps.tile([HD + 1, QC], F32, tag="pj", name=f"cxs1_{p}_{j}")

                def ctx_self_mm(kcc):
                    for h01, cx in ((0, dx0), (1, dx1)):
                        nc.tensor.matmul(
                            cx[:],
                            vaug[:, kcc, 2 * p + h01, :],
                            ex_sl(ex, kcc)[:, h01, :],
                            start=(kcc == 0),
                            stop=(kcc == KCH - 1),
                        )

                # kc-PAIR clustering: both score pairs (64-row-tiled mode)
                # back to back, then all 128-row work (proj weave + ctx) for
                # the pair — 2 PE mode-switch drains per 2 kcs instead of 4
                for kcp in range(KCH // 2):
                    kc0, kc1 = 2 * kcp, 2 * kcp + 1
                    scores_exp_part(p, j, ex, [kc0])
                    scores_exp_part(p, j, ex, [kc1])
                    weave(gens, budget)
                    for kc in (kc0, kc1):
                        if cp is not None:
                            for h01, cx in ((0, cx0), (1, cx1)):
                                nc.tensor.matmul(
                                    cx[:],
                                    vaug[:, kc, 2 * cp + h01, :],
                                    ex_sl(cex, kc)[:, h01, :],
                                    start=(kc == 0),
                                    stop=(kc == KCH - 1),
                                )
                        if ctx2 is not None:
                            dp, dj, dex = ctx2
                            for h01, cx in ((0, dx0), (1, dx1)):
                                nc.tensor.matmul(
                                    cx[:],
                                    vaug[:, kc, 2 * dp + h01, :],
                                    ex_sl(dex, kc)[:, h01, :],
                                    start=(kc == 0),
                                    stop=(kc == KCH - 1),
                                )
                    if ctx_self and kcp >= 1:
                        ctx_self_mm(kc0 - 2)
                        ctx_self_mm(kc1 - 2)
                drain(gens)
                if ctx_self:
                    ctx_self_mm(KCH - 2)
                    ctx_self_mm(KCH - 1)
                if cp is not None:
                    ctx_evac(cp, cj, cx0, cx1)
                if ctx2 is not None:
                    dp, dj, _ = ctx2
                    ctx_evac(dp, dj, dx0, dx1)
                if ctx_self:
                    ctx_evac(p, j, dx0, dx1, on_vector=True)
                return ex

            # Pair 0 is special-ordered so ACT starts as early as possible:
            # scores/exp need only qT/kT; v-projection matmuls fill PE gaps
            # while ACT chews exps; ctx comes after proj_v (vaug dependency).
            # Next pair's projection chains are spread through the current
            # pair's attention units so the scheduler can hide them in the
            # ACT-gated gaps instead of paying for them at pair boundaries.
            # Startup: first kT/qT chains run as blocks (critical path),
            # then everything — remaining chains, the v projection, next
            # pair's chains — is woven between score pairs at kc grain.
            drain([chain_gen(0, "wk", 0)])
            drain([chain_gen(0, "wq", 0)])
            ex00 = alloc_ex(0, 0)
            g = [chain_gen(0, "wk", 1), chain_gen(0, "wk", 2),
                 chain_gen(0, "wk", 3), chain_gen(0, "wq", 1)]
            for kcp in range(KCH // 2):
                scores_exp_part(0, 0, ex00, [2 * kcp])
                scores_exp_part(0, 0, ex00, [2 * kcp + 1])
                weave(g, 4)
            drain(g)
            ex01 = alloc_ex(0, 1)
            g = [chain_gen(0, "wq", 2)] + [v_gen(kb) for kb in range(KCH)]
            for kcp in range(KCH // 2):
                scores_exp_part(0, 1, ex01, [2 * kcp])
                scores_exp_part(0, 1, ex01, [2 * kcp + 1])
                weave(g, 6)
            drain(g)  # ctx(0,0) in the next unit needs all of v
            ex02 = fused_unit(0, 2, 0, 0, ex00, [chain_gen(0, "wq", 3)])
            ex03 = fused_unit(
                0, 3, 0, 1, ex01,
                [chain_gen(1, "wk", 0), chain_gen(1, "wk", 1), chain_gen(1, "wq", 0)],
            )
            ex10 = fused_unit(
                1, 0, 0, 2, ex02,
                [chain_gen(1, "wk", 2), chain_gen(1, "wk", 3), chain_gen(1, "wq", 1)],
            )
            ex11 = fused_unit(
                1, 1, 0, 3, ex03, [chain_gen(1, "wq", 2), chain_gen(2, "wk", 0)]
            )
            ex12 = fused_unit(
                1, 2, 1, 0, ex10, [chain_gen(1, "wq", 3), chain_gen(2, "wk", 1)]
            )
            ex13 = fused_unit(
                1, 3, 1, 1, ex11, [chain_gen(2, "wk", 2), chain_gen(2, "wq", 0)]
            )
            ex20 = fused_unit(
                2, 0, 1, 2, ex12, [chain_gen(2, "wk", 3), chain_gen(2, "wq", 1)]
            )
            ex21 = fused_unit(
                2, 1, 1, 3, ex13, [chain_gen(2, "wq", 2), chain_gen(2, "wq", 3)]
            )
            # tail shrink: the last two units each carry TWO ctx units —
            # one on the cx banks, one on the now-idle proj banks; the very
            # last unit self-consumes its own ex at a 2-kc lag so only two
            # ctx chunks trail the final scores
            ex22 = fused_unit(2, 2, 2, 0, ex20, ctx2=(2, 1, ex21))
            ex23 = fused_unit(2, 3, 2, 2, ex22, ctx_self=True)

    nc.compile()
    return nc


def _get_nc():
    if "nc" not in _NC_CACHE:
        _NC_CACHE["nc"] = _build_nc()
    return _NC_CACHE["nc"]


def _make_in_maps(hidden, mask, Wq, Wk, Wv):
    bf16 = ml_dtypes.bfloat16
    in_maps = []
    for c in range(N_CORES):
        b, hg = c // 2, c % 2
        cols = slice(hg * HEADS_PER_CORE * HD, (hg + 1) * HEADS_PER_CORE * HD)
        mc = np.ascontiguousarray(
            mask[b, 0, 0].astype(np.float32).reshape(KCH, 128).T
        )
        in_maps.append(
            {
                "ht": np.ascontiguousarray(hidden[b].T).astype(bf16),
                "wq": np.ascontiguousarray(Wq[:, cols]).astype(bf16),
                "wk": np.ascontiguousarray(Wk[:, cols]).astype(bf16),
                "wv": np.ascontiguousarray(Wv[:, cols]).astype(bf16),
                "mask": mc,
            }
        )
    return in_maps


def _gather(results):
    out = np.empty((B, S, H), dtype=np.float32)
    for c in range(N_CORES):
        b, hg = c // 2, c % 2
        r = results[c]["out"]  # [6, 65, S]
        num = r[:, :HD, :]  # [6, 64, S]
        den = r[:, HD : HD + 1, :]  # [6, 1, S]
        ctx = np.transpose(num / den, (2, 0, 1)).reshape(S, HEADS_PER_CORE * HD)
        out[b, :, hg * HEADS_PER_CORE * HD : (hg + 1) * HEADS_PER_CORE * HD] = ctx
    return out


def _run_device(hidden, mask, Wq, Wk, Wv, trace=False):
    nc = _get_nc()
    in_maps = _make_in_maps(hidden, mask, Wq, Wk, Wv)
    res = run_bass_kernel_spmd(nc, in_maps, core_ids=list(range(N_CORES)), trace=trace)
    return _gather(res.results), res


def _numpy_fallback(hidden_states, attention_mask, Wq, bq, Wk, bk, Wv, bv):
    def split_heads(x):
        return x.reshape(B, S, NH, HD).transpose(0, 2, 1, 3)

    q = split_heads(hidden_states @ Wq + bq)
    k = split_heads(hidden_states @ Wk + bk)
    v = split_heads(hidden_states @ Wv + bv)
    scores = np.einsum("bhqd,bhkd->bhqk", q, k) / np.sqrt(HD) + attention_mask
    scores -= scores.max(axis=-1, keepdims=True)
    e = np.exp(scores)
    probs = e / e.sum(axis=-1, keepdims=True)
    ctx = np.einsum("bhqk,bhkd->bhqd", probs, v)
    return ctx.transpose(0, 2, 1, 3).reshape(B, S, H).astype(np.float32)


def kernel(hidden_states, attention_mask, Wq, bq, Wk, bk, Wv, bv):
    hidden = np.asarray(hidden_states, dtype=np.float32)
    mask = np.asarray(attention_mask, dtype=np.float32)
    Wq = np.asarray(Wq, dtype=np.float32)
    Wk = np.asarray(Wk, dtype=np.float32)
    Wv = np.asarray(Wv, dtype=np.float32)
    bq, bk, bv = (np.asarray(x, dtype=np.float32) for x in (bq, bk, bv))
    if np.any(bq) or np.any(bk) or np.any(bv):
        # projection biases are zero for this problem; keep a correct
        # fallback rather than a dead device path
        return _numpy_fallback(hidden, mask, Wq, bq, Wk, bk, Wv, bv)
    out, _ = _run_device(hidden, mask, Wq, Wk, Wv)
    return out



# revision 14
# speedup vs baseline: 1.0274x; 1.0112x over previous
"""BertSelfAttention on 8 TRN2 NeuronCores.

B=4, S=2048, H=768, NH=12, HD=64. Sharding: core c <- (batch c//2,
head-group c%2 of 6 heads). No collectives; host shards/gathers.

Device kernel (per core, bf16 matmuls / fp32 PSUM):
  - qT/kT projections: weights stationary -> [2*64 dims, S] per head pair
  - v projection: hiddenT stationary -> natural [s, d] layout + ones column
  - scoresT = kT.T-major: scores computed transposed [k, q] so the exp'd
    probs feed the ctx matmul directly (contraction k on partitions);
    K=64 contraction row-packed 2 heads per pass via tile_position
  - softmax without max-subtraction (scores ~ N(0,1)); exp on ACT with
    scale=1/8 and mask as per-partition bias; denominator = ones column
  - output per head: [65, S] = unnormalized ctxT + denominator row;
    host divides + transposes during gather
  - kc-PAIR clustering: both score pairs (64-row-tiled PE mode) emit back
    to back, then all 128-row work (ctx + woven proj) for the pair: the
    64<->128-row retile drain (~105ns each way) is paid once per 2 kcs
    instead of per kc; steady-state MMs then run at pure stream time
    (~216ns for N=512 bf16) with LDWEIGHTS fully hidden
  - tried and REVERTED (both regressed): finer load-phase striping of
    wk/wq pair-slices (+6us — extra dma_start issues delay the queues)
    and splitting the final unit's evac DMAs into column halves (+2.6us)
"""

import sys

sys.path.insert(0, "/opt/trn_rl_repo")

import numpy as np
import ml_dtypes

import concourse.bacc as bacc
import concourse.mybir as mybir
import concourse.tile as tile
from concourse.bass_utils import run_bass_kernel_spmd

B, S, H, NH, HD = 4, 2048, 768, 12, 64
N_CORES = 8
HEADS_PER_CORE = NH // 2  # 6
N_PAIR = HEADS_PER_CORE // 2  # 3
CCH = H // 128  # 6 contraction chunks
QC = 512  # q chunk width (moving dim)
QCH = S // QC  # 4
KCH = S // 128  # 16 k chunks
SCALE = 1.0 / float(np.sqrt(HD))

BF16 = mybir.dt.bfloat16
F32 = mybir.dt.float32
I16 = mybir.dt.int16

# DVE-offloaded exp: ex = 2^(t) with t = s*SCALE*log2e + mask*log2e,
# approximated in bf16 bit space (Schraudolph): bits16 = floor(t*128 +
# (127-C)*128 + mask*128*log2e + 0.5). One tensor_scalar per kc group
# (mult+add, f32 PSUM -> int16 SBUF), consumed via bitcast as bf16.
# Softmax num/den share the approx error, so it largely cancels; host
# sim: 6/16 groups offloaded -> rel err ~9e-3 (gate 2e-2).
LOG2E = 1.4426950408889634
A16 = 128.0 * LOG2E * SCALE
B16 = 128.0 * (127.0 - 0.0573) + 0.5
# even kcs on DVE: strict ACT/DVE alternation (incl. across unit
# boundaries) keeps both consumers ahead of the PE's score-pair cadence
# (all-core host sim @ C=0.0573: ~1.2e-2 < 2e-2 gate; odd-kc sets
# measured worse on device)
DVE_KC = (0, 2, 4, 6, 8, 10, 12, 14)

_NC_CACHE = {}


def _build_nc():
    nc = bacc.Bacc("TRN2", target_bir_lowering=False)

    ht_ext = nc.dram_tensor("ht", [H, S], BF16, kind="ExternalInput")
    wq_ext = nc.dram_tensor("wq", [H, HEADS_PER_CORE * HD], BF16, kind="ExternalInput")
    wk_ext = nc.dram_tensor("wk", [H, HEADS_PER_CORE * HD], BF16, kind="ExternalInput")
    wv_ext = nc.dram_tensor("wv", [H, HEADS_PER_CORE * HD], BF16, kind="ExternalInput")
    mask_ext = nc.dram_tensor("mask", [128, KCH], F32, kind="ExternalInput")
    out_ext = nc.dram_tensor(
        "out", [HEADS_PER_CORE, HD + 1, S], F32, kind="ExternalOutput"
    )

    with tile.TileContext(nc) as tc:
        with (
            tc.tile_pool(name="const", bufs=1) as const,
            tc.tile_pool(name="qk", bufs=1) as qk,
            tc.tile_pool(name="expp", bufs=4) as expp,
            tc.tile_pool(name="outp", bufs=3) as outp,
            tc.tile_pool(name="pj_ps", bufs=2, space="PSUM") as pj_ps,
            tc.tile_pool(name="sc_ps", bufs=2, space="PSUM") as sc_ps,
            # two tags (cx0/cx1) x bufs=1 -> 2 banks
            tc.tile_pool(name="cx_ps", bufs=1, space="PSUM") as cx_ps,
        ):
            # ---- loads, ordered for the first kT/qT chains' critical path.
            # dma_start issue cost scales with partition rows (~0.7us per
            # 128-row transfer on the issuing queue), so the critical path
            # is (a) queue-issue serialization and (b) the first transfers'
            # bytes. The first kt chain needs ONLY the pair-0 column slice
            # of wk (cols 0:128) plus ht q0 — load exactly that first,
            # striped over FOUR queues (scalar+vector are idle until the
            # exp stream starts ~20us in).
            mask_sb = const.tile([128, KCH], F32, tag="mask")
            m16 = const.tile([128, KCH], F32, tag="m16")
            w_sb = {
                name: const.tile(
                    [128, CCH, HEADS_PER_CORE * HD], BF16, tag=name, name=name
                )
                for name in ("wq", "wk", "wv")
            }
            ht_sb = const.tile([128, CCH, S], BF16, tag="ht")

            def _w(name, ext, cc, eng):
                eng.dma_start(
                    out=w_sb[name][:, cc, :],
                    in_=ext[cc * 128 : (cc + 1) * 128, :],
                )

            def _wp(name, ext, cc, p, eng):
                eng.dma_start(
                    out=w_sb[name][:, cc, p * 128 : (p + 1) * 128],
                    in_=ext[cc * 128 : (cc + 1) * 128, p * 128 : (p + 1) * 128],
                )

            def _hq(cc, q, eng):
                eng.dma_start(
                    out=ht_sb[:, cc, q * QC : (q + 1) * QC],
                    in_=ht_ext[cc * 128 : (cc + 1) * 128, q * QC : (q + 1) * QC],
                )

            # mask first on sync (exp bias + ACT warm dependency)
            nc.sync.dma_start(out=mask_sb[:], in_=mask_ext[:])
            # per-partition bias for the DVE exp groups: mask*128*log2e + B16
            nc.vector.tensor_scalar(
                out=m16[:],
                in0=mask_sb[:],
                scalar1=128.0 * LOG2E,
                scalar2=B16,
                op0=mybir.AluOpType.mult,
                op1=mybir.AluOpType.add,
            )
            # first chains (kt/qt pair0 sch0) need only wk+wq and ht q-chunk
            # 0: load that critical set first, striped over THREE queues
            # (scalar is idle during the load phase; 3 queues ~ HBM cap),
            # cc-interleaved so the woven chain matmuls start as chunks land
            q3 = [nc.sync, nc.gpsimd, nc.scalar]
            for cc in range(CCH):
                _w("wk", wk_ext, cc, q3[cc % 3])
                _w("wq", wq_ext, cc, q3[(cc + 1) % 3])
                _hq(cc, 0, q3[(cc + 2) % 3])
            # dummy 1-elem exp after scalar's critical-DMA share: pulls the
            # ~2.7us ACT table load into the load phase, before the first
            # real score activation
            warm = const.tile([128, 1], F32, tag="warm")
            nc.scalar.activation(
                out=warm[:],
                in_=mask_sb[:, 0:1],
                func=mybir.ActivationFunctionType.Exp,
            )
            # non-critical remainder on sync/gpsimd only (scalar must stay
            # clear for the exp stream): ht q1 (chains sch1) and wv (v
            # proj, woven from unit (0,1)) trickle in PARALLEL on separate
            # queues so neither consumer stalls, then ht q2/q3
            for cc in range(CCH):
                _hq(cc, 1, nc.sync)
                _w("wv", wv_ext, cc, nc.gpsimd)
            for q in range(2, QCH):
                for cc in range(CCH):
                    _hq(cc, q, nc.sync if (cc + q) % 2 == 0 else nc.gpsimd)

            # v_aug[p, kc, h, 0:64] = v, [..., 64] = 1.0 (denominator column)
            vaug = const.tile([128, KCH, HEADS_PER_CORE, HD + 1], BF16, tag="vaug")
            nc.vector.memset(vaug[:, :, :, HD : HD + 1], 1.0)

            qt_sb = [
                qk.tile([128, S], BF16, name=f"qt{p}", tag=f"qt{p}")
                for p in range(N_PAIR)
            ]
            kt_sb = [
                qk.tile([128, S], BF16, name=f"kt{p}", tag=f"kt{p}")
                for p in range(N_PAIR)
            ]

            def chain_gen(p, name, sch):
                # lazy emission of one qT/kT projection chain: one
                # instruction per next() so it can be woven between score
                # pairs (the PE queue is strict FIFO)
                w = w_sb[name]
                dst = (qt_sb if name == "wq" else kt_sb)[p]
                ps = pj_ps.tile([128, QC], F32, tag="pj", name=f"pj_{name}{p}_{sch}")
                for cc in range(CCH):
                    nc.tensor.matmul(
                        ps[:],
                        w[:, cc, p * 128 : (p + 1) * 128],
                        ht_sb[:, cc, sch * QC : (sch + 1) * QC],
                        start=(cc == 0),
                        stop=(cc == CCH - 1),
                    )
                    yield
                nc.vector.tensor_copy(dst[:, sch * QC : (sch + 1) * QC], ps[:])
                yield

            def v_gen(kb):
                # one k-block of the v projection (all 6 heads)
                wv = w_sb["wv"]
                ps = pj_ps.tile(
                    [128, HEADS_PER_CORE * HD], F32, tag="pj", name=f"pv_{kb}"
                )
                for cc in range(CCH):
                    nc.tensor.matmul(
                        ps[:],
                        ht_sb[:, cc, kb * 128 : (kb + 1) * 128],
                        wv[:, cc, :],
                        start=(cc == 0),
                        stop=(cc == CCH - 1),
                    )
                    yield
                nc.vector.tensor_copy(
                    vaug[:, kb, :, 0:HD],
                    ps[:].rearrange("p (h d) -> p h d", h=HEADS_PER_CORE),
                )
                yield

            def weave(gens, budget):
                n = 0
                while gens and n < budget:
                    try:
                        next(gens[0])
                        n += 1
                    except StopIteration:
                        gens.pop(0)

            def drain(gens):
                while gens:
                    try:
                        next(gens[0])
                    except StopIteration:
                        gens.pop(0)

            def alloc_ex(p, j):
                return expp.tile([128, KCH, 2, QC], BF16, tag="ex", name=f"ex{p}_{j}")

            def ex_sl(ex, kc):
                return ex[:, kc]

            def scores_exp_part(p, j, ex, kcs):
                for kc in kcs:
                    sc = sc_ps.tile([128, 2, QC], F32, tag="sc", name=f"sc{p}_{j}_{kc}")
                    for h01 in range(2):
                        lo, hi = h01 * 64, h01 * 64 + 64
                        nc.tensor.matmul(
                            sc[:, h01, :],
                            kt_sb[p][lo:hi, kc * 128 : (kc + 1) * 128],
                            qt_sb[p][lo:hi, j * QC : (j + 1) * QC],
                            start=True,
                            stop=True,
                        )
                    if kc in DVE_KC:
                        nc.vector.tensor_scalar(
                            out=ex_sl(ex, kc).bitcast(I16),
                            in0=sc[:],
                            scalar1=A16,
                            scalar2=m16[:, kc : kc + 1],
                            op0=mybir.AluOpType.mult,
                            op1=mybir.AluOpType.add,
                        )
                    else:
                        nc.scalar.activation(
                            out=ex_sl(ex, kc),
                            in_=sc[:],
                            func=mybir.ActivationFunctionType.Exp,
                            scale=SCALE,
                            bias=mask_sb[:, kc : kc + 1],
                        )

            def scores_exp(p, j):
                ex = alloc_ex(p, j)
                scores_exp_part(p, j, ex, range(KCH))
                return ex

            def ctx_evac(cp, cj, cx0, cx1, on_vector=False):
                # final unit: DMAs ride sync+scalar so the gpsimd queue's
                # last DMA lands a unit earlier and its slow SWDGE teardown
                # drain (~3.6us) overlaps the tail instead of trailing it
                eng1 = nc.scalar if on_vector else nc.gpsimd
                for h01, cx, eng in ((0, cx0, nc.sync), (1, cx1, eng1)):
                    o_sb = outp.tile(
                        [HD + 1, QC], F32, tag="o", name=f"o{cp}_{cj}_{h01}"
                    )
                    # scalar engine (has slack) evacuates ctx psum so the
                    # vector queue stays clear for exp groups + proj casts;
                    # the final self-ctx unit uses the (by then idle)
                    # vector queue so the two tail evacs run in parallel
                    if on_vector:
                        nc.vector.tensor_copy(o_sb[:], cx[:])
                    else:
                        nc.scalar.copy(o_sb[:], cx[:])
                    eng.dma_start(
                        out=out_ext[2 * cp + h01, :, cj * QC : (cj + 1) * QC],
                        in_=o_sb[:],
                    )

            def fused_unit(p, j, cp, cj, cex, gens=(), budget=4, ctx2=None,
                           ctx_self=None):
                # scores+exp of unit (p, j) interleaved per-kc with the ctx
                # accumulation of unit (cp, cj) and woven projection chain
                # steps: the PE engine queue is strict FIFO, so emission
                # order must match the steady-state demand (one score pair
                # per ACT period, ctx + proj fill the rest) or ACT starves.
                gens = list(gens)
                ex = alloc_ex(p, j)
                cx0 = cx1 = None
                if cp is not None:
                    cx0 = cx_ps.tile([HD + 1, QC], F32, tag="cx0", name=f"cx0_{cp}_{cj}")
                    cx1 = cx_ps.tile([HD + 1, QC], F32, tag="cx1", name=f"cx1_{cp}_{cj}")
                dx0 = dx1 = None
                if ctx2 is not None:
                    dp, dj, _ = ctx2
                    dx0 = pj_ps.tile([HD + 1, QC], F32, tag="pj", name=f"cx0_{dp}_{dj}")
                    dx1 = pj_ps.tile([HD + 1, QC], F32, tag="pj", name=f"cx1_{dp}_{dj}")
                if ctx_self:
                    # this unit's own ctx rides the pj banks, consuming its
                    # own ex with a 2-kc lag (the exp of kc is ready by the
                    # time the PE reaches kc+2) — kills the serial PE tail
                    dx0 = pj_ps.tile([HD + 1, QC], F32, tag="pj", name=f"cxs0_{p}_{j}")
                    dx1 = pj_ps.tile([HD + 1, QC], F32, tag="pj", name=f"cxs1_{p}_{j}")

                def ctx_self_mm(kcc):
                    for h01, cx in ((0, dx0), (1, dx1)):
                        nc.tensor.matmul(
                            cx[:],
                            vaug[:, kcc, 2 * p + h01, :],
                            ex_sl(ex, kcc)[:, h01, :],
                            start=(kcc == 0),
                            stop=(kcc == KCH - 1),
                        )

                # kc-PAIR clustering: both score pairs (64-row-tiled mode)
                # back to back, then all 128-row work (proj weave + ctx) for
                # the pair — 2 PE mode-switch drains per 2 kcs instead of 4
                for kcp in range(KCH // 2):
                    kc0, kc1 = 2 * kcp, 2 * kcp + 1
                    scores_exp_part(p, j, ex, [kc0])
                    scores_exp_part(p, j, ex, [kc1])
                    weave(gens, budget)
                    for kc in (kc0, kc1):
                        if cp is not None:
                            for h01, cx in ((0, cx0), (1, cx1)):
                                nc.tensor.matmul(
                                    cx[:],
                                    vaug[:, kc, 2 * cp + h01, :],
                                    ex_sl(cex, kc)[:, h01, :],
                                    start=(kc == 0),
                                    stop=(kc == KCH - 1),
                                )
                        if ctx2 is not None:
                            dp, dj, dex = ctx2
                            for h01, cx in ((0, dx0), (1, dx1)):
                                nc.tensor.matmul(
                                    cx[:],
                                    vaug[:, kc, 2 * dp + h01, :],
                                    ex_sl(dex, kc)[:, h01, :],
                                    start=(kc == 0),
                                    stop=(kc == KCH - 1),
                                )
                    if ctx_self and kcp >= 1:
                        ctx_self_mm(kc0 - 2)
                        ctx_self_mm(kc1 - 2)
                drain(gens)
                if ctx_self:
                    ctx_self_mm(KCH - 2)
                    ctx_self_mm(KCH - 1)
                if cp is not None:
                    ctx_evac(cp, cj, cx0, cx1)
                if ctx2 is not None:
                    dp, dj, _ = ctx2
                    ctx_evac(dp, dj, dx0, dx1)
                if ctx_self:
                    ctx_evac(p, j, dx0, dx1, on_vector=True)
                return ex

            # Pair 0 is special-ordered so ACT starts as early as possible:
            # scores/exp need only qT/kT; v-projection matmuls fill PE gaps
            # while ACT chews exps; ctx comes after proj_v (vaug dependency).
            # Next pair's projection chains are spread through the current
            # pair's attention units so the scheduler can hide them in the
            # ACT-gated gaps instead of paying for them at pair boundaries.
            # Startup: first kT/qT chains run as blocks (critical path),
            # then everything — remaining chains, the v projection, next
            # pair's chains — is woven between score pairs at kc grain.
            drain([chain_gen(0, "wk", 0)])
            drain([chain_gen(0, "wq", 0)])
            ex00 = alloc_ex(0, 0)
            g = [chain_gen(0, "wk", 1), chain_gen(0, "wk", 2),
                 chain_gen(0, "wk", 3), chain_gen(0, "wq", 1)]
            for kcp in range(KCH // 2):
                scores_exp_part(0, 0, ex00, [2 * kcp])
                scores_exp_part(0, 0, ex00, [2 * kcp + 1])
                weave(g, 4)
            drain(g)
            ex01 = alloc_ex(0, 1)
            g = [chain_gen(0, "wq", 2)] + [v_gen(kb) for kb in range(KCH)]
            for kcp in range(KCH // 2):
                scores_exp_part(0, 1, ex01, [2 * kcp])
                scores_exp_part(0, 1, ex01, [2 * kcp + 1])
                weave(g, 6)
            drain(g)  # ctx(0,0) in the next unit needs all of v
            ex02 = fused_unit(0, 2, 0, 0, ex00, [chain_gen(0, "wq", 3)])
            ex03 = fused_unit(
                0, 3, 0, 1, ex01,
                [chain_gen(1, "wk", 0), chain_gen(1, "wk", 1), chain_gen(1, "wq", 0)],
            )
            ex10 = fused_unit(
                1, 0, 0, 2, ex02,
                [chain_gen(1, "wk", 2), chain_gen(1, "wk", 3), chain_gen(1, "wq", 1)],
            )
            ex11 = fused_unit(
                1, 1, 0, 3, ex03, [chain_gen(1, "wq", 2), chain_gen(2, "wk", 0)]
            )
            ex12 = fused_unit(
                1, 2, 1, 0, ex10, [chain_gen(1, "wq", 3), chain_gen(2, "wk", 1)]
            )
            ex13 = fused_unit(
                1, 3, 1, 1, ex11, [chain_gen(2, "wk", 2), chain_gen(2, "wq", 0)]
            )
            ex20 = fused_unit(
                2, 0, 1, 2, ex12, [chain_gen(2, "wk", 3), chain_gen(2, "wq", 1)]
            )
            ex21 = fused_unit(
                2, 1, 1, 3, ex13, [chain_gen(2, "wq", 2), chain_gen(2, "wq", 3)]
            )
            # tail shrink: the last two units each carry TWO ctx units —
            # one on the cx banks, one on the now-idle proj banks; the very
            # last unit self-consumes its own ex at a 2-kc lag so only two
            # ctx chunks trail the final scores
            ex22 = fused_unit(2, 2, 2, 0, ex20, ctx2=(2, 1, ex21))
            ex23 = fused_unit(2, 3, 2, 2, ex22, ctx_self=True)

    nc.compile()
    return nc


def _get_nc():
    if "nc" not in _NC_CACHE:
        _NC_CACHE["nc"] = _build_nc()
    return _NC_CACHE["nc"]


def _make_in_maps(hidden, mask, Wq, Wk, Wv):
    bf16 = ml_dtypes.bfloat16
    in_maps = []
    for c in range(N_CORES):
        b, hg = c // 2, c % 2
        cols = slice(hg * HEADS_PER_CORE * HD, (hg + 1) * HEADS_PER_CORE * HD)
        mc = np.ascontiguousarray(
            mask[b, 0, 0].astype(np.float32).reshape(KCH, 128).T
        )
        in_maps.append(
            {
                "ht": np.ascontiguousarray(hidden[b].T).astype(bf16),
                "wq": np.ascontiguousarray(Wq[:, cols]).astype(bf16),
                "wk": np.ascontiguousarray(Wk[:, cols]).astype(bf16),
                "wv": np.ascontiguousarray(Wv[:, cols]).astype(bf16),
                "mask": mc,
            }
        )
    return in_maps


def _gather(results):
    out = np.empty((B, S, H), dtype=np.float32)
    for c in range(N_CORES):
        b, hg = c // 2, c % 2
        r = results[c]["out"]  # [6, 65, S]
        num = r[:, :HD, :]  # [6, 64, S]
        den = r[:, HD : HD + 1, :]  # [6, 1, S]
        ctx = np.transpose(num / den, (2, 0, 1)).reshape(S, HEADS_PER_CORE * HD)
        out[b, :, hg * HEADS_PER_CORE * HD : (hg + 1) * HEADS_PER_CORE * HD] = ctx
    return out


def _run_device(hidden, mask, Wq, Wk, Wv, trace=False):
    nc = _get_nc()
    in_maps = _make_in_maps(hidden, mask, Wq, Wk, Wv)
    res = run_bass_kernel_spmd(nc, in_maps, core_ids=list(range(N_CORES)), trace=trace)
    return _gather(res.results), res


def _numpy_fallback(hidden_states, attention_mask, Wq, bq, Wk, bk, Wv, bv):
    def split_heads(x):
        return x.reshape(B, S, NH, HD).transpose(0, 2, 1, 3)

    q = split_heads(hidden_states @ Wq + bq)
    k = split_heads(hidden_states @ Wk + bk)
    v = split_heads(hidden_states @ Wv + bv)
    scores = np.einsum("bhqd,bhkd->bhqk", q, k) / np.sqrt(HD) + attention_mask
    scores -= scores.max(axis=-1, keepdims=True)
    e = np.exp(scores)
    probs = e / e.sum(axis=-1, keepdims=True)
    ctx = np.einsum("bhqk,bhkd->bhqd", probs, v)
    return ctx.transpose(0, 2, 1, 3).reshape(B, S, H).astype(np.float32)


def kernel(hidden_states, attention_mask, Wq, bq, Wk, bk, Wv, bv):
    hidden = np.asarray(hidden_states, dtype=np.float32)
    mask = np.asarray(attention_mask, dtype=np.float32)
    Wq = np.asarray(Wq, dtype=np.float32)
    Wk = np.asarray(Wk, dtype=np.float32)
    Wv = np.asarray(Wv, dtype=np.float32)
    bq, bk, bv = (np.asarray(x, dtype=np.float32) for x in (bq, bk, bv))
    if np.any(bq) or np.any(bk) or np.any(bv):
        # projection biases are zero for this problem; keep a correct
        # fallback rather than a dead device path
        return _numpy_fallback(hidden, mask, Wq, bq, Wk, bk, Wv, bv)
    out, _ = _run_device(hidden, mask, Wq, Wk, Wv)
    return out



# revision 16
# speedup vs baseline: 1.0395x; 1.0118x over previous
"""BertSelfAttention on 8 TRN2 NeuronCores.

B=4, S=2048, H=768, NH=12, HD=64. Sharding: core c <- (batch c//2,
head-group c%2 of 6 heads). No collectives; host shards/gathers.

Device kernel (per core, bf16 matmuls / fp32 PSUM):
  - qT/kT projections: weights stationary -> [2*64 dims, S] per head pair
  - v projection: hiddenT stationary -> natural [s, d] layout + ones column
  - scoresT = kT.T-major: scores computed transposed [k, q] so the exp'd
    probs feed the ctx matmul directly (contraction k on partitions);
    K=64 contraction row-packed 2 heads per pass via tile_position
  - softmax without max-subtraction (scores ~ N(0,1)); exp on ACT with
    scale=1/8 and mask as per-partition bias; denominator = ones column
  - output per head: [65, S] = unnormalized ctxT + denominator row;
    host divides + transposes during gather
  - kc-PAIR clustering: both score pairs (64-row-tiled PE mode) emit back
    to back, then all 128-row work (ctx + woven proj) for the pair: the
    64<->128-row retile drain (~105ns each way) is paid once per 2 kcs
    instead of per kc; steady-state MMs then run at pure stream time
    (~216ns for N=512 bf16) with LDWEIGHTS fully hidden
  - tried and REVERTED (both regressed): finer load-phase striping of
    wk/wq pair-slices (+6us — extra dma_start issues delay the queues)
    and splitting the final unit's evac DMAs into column halves (+2.6us)
"""

import sys

sys.path.insert(0, "/opt/trn_rl_repo")

import numpy as np
import ml_dtypes

import concourse.bacc as bacc
import concourse.mybir as mybir
import concourse.tile as tile
from concourse.bass_utils import run_bass_kernel_spmd

B, S, H, NH, HD = 4, 2048, 768, 12, 64
N_CORES = 8
HEADS_PER_CORE = NH // 2  # 6
N_PAIR = HEADS_PER_CORE // 2  # 3
CCH = H // 128  # 6 contraction chunks
QC = 512  # q chunk width (moving dim)
QCH = S // QC  # 4
KCH = S // 128  # 16 k chunks
SCALE = 1.0 / float(np.sqrt(HD))

BF16 = mybir.dt.bfloat16
F32 = mybir.dt.float32
I16 = mybir.dt.int16

# DVE-offloaded exp: ex = 2^(t) with t = s*SCALE*log2e + mask*log2e,
# approximated in bf16 bit space (Schraudolph): bits16 = floor(t*128 +
# (127-C)*128 + mask*128*log2e + 0.5). One tensor_scalar per kc group
# (mult+add, f32 PSUM -> int16 SBUF), consumed via bitcast as bf16.
# Softmax num/den share the approx error, so it largely cancels; host
# sim: 6/16 groups offloaded -> rel err ~9e-3 (gate 2e-2).
LOG2E = 1.4426950408889634
A16 = 128.0 * LOG2E * SCALE
B16 = 128.0 * (127.0 - 0.0573) + 0.5
# even kcs on DVE: strict ACT/DVE alternation (incl. across unit
# boundaries) keeps both consumers ahead of the PE's score-pair cadence
# (all-core host sim @ C=0.0573: ~1.2e-2 < 2e-2 gate; odd-kc sets
# measured worse on device)
DVE_KC = (0, 2, 4, 6, 8, 10, 12, 14)

_NC_CACHE = {}


def _build_nc():
    nc = bacc.Bacc("TRN2", target_bir_lowering=False)

    ht_ext = nc.dram_tensor("ht", [H, S], BF16, kind="ExternalInput")
    wq_ext = nc.dram_tensor("wq", [H, HEADS_PER_CORE * HD], BF16, kind="ExternalInput")
    wk_ext = nc.dram_tensor("wk", [H, HEADS_PER_CORE * HD], BF16, kind="ExternalInput")
    wv_ext = nc.dram_tensor("wv", [H, HEADS_PER_CORE * HD], BF16, kind="ExternalInput")
    mask_ext = nc.dram_tensor("mask", [128, KCH], F32, kind="ExternalInput")
    out_ext = nc.dram_tensor(
        "out", [HEADS_PER_CORE, HD + 1, S], F32, kind="ExternalOutput"
    )

    with tile.TileContext(nc) as tc:
        with (
            tc.tile_pool(name="const", bufs=1) as const,
            tc.tile_pool(name="qk", bufs=1) as qk,
            tc.tile_pool(name="expp", bufs=4) as expp,
            tc.tile_pool(name="outp", bufs=3) as outp,
            tc.tile_pool(name="pj_ps", bufs=2, space="PSUM") as pj_ps,
            tc.tile_pool(name="sc_ps", bufs=2, space="PSUM") as sc_ps,
            # two tags (cx0/cx1) x bufs=1 -> 2 banks
            tc.tile_pool(name="cx_ps", bufs=1, space="PSUM") as cx_ps,
        ):
            # ---- loads, ordered for the first kT/qT chains' critical path.
            # dma_start issue cost scales with partition rows (~0.7us per
            # 128-row transfer on the issuing queue), so the critical path
            # is (a) queue-issue serialization and (b) the first transfers'
            # bytes. The first kt chain needs ONLY the pair-0 column slice
            # of wk (cols 0:128) plus ht q0 — load exactly that first,
            # striped over FOUR queues (scalar+vector are idle until the
            # exp stream starts ~20us in).
            mask_sb = const.tile([128, KCH], F32, tag="mask")
            m16 = const.tile([128, KCH], F32, tag="m16")
            w_sb = {
                name: const.tile(
                    [128, CCH, HEADS_PER_CORE * HD], BF16, tag=name, name=name
                )
                for name in ("wq", "wk", "wv")
            }
            ht_sb = const.tile([128, CCH, S], BF16, tag="ht")

            def _w(name, ext, cc, eng):
                eng.dma_start(
                    out=w_sb[name][:, cc, :],
                    in_=ext[cc * 128 : (cc + 1) * 128, :],
                )

            def _wp(name, ext, cc, p, eng):
                eng.dma_start(
                    out=w_sb[name][:, cc, p * 128 : (p + 1) * 128],
                    in_=ext[cc * 128 : (cc + 1) * 128, p * 128 : (p + 1) * 128],
                )

            def _hq(cc, q, eng):
                eng.dma_start(
                    out=ht_sb[:, cc, q * QC : (q + 1) * QC],
                    in_=ht_ext[cc * 128 : (cc + 1) * 128, q * QC : (q + 1) * QC],
                )

            # mask on gpsimd (exp bias + ACT warm dependency): off the sync
            # queue so wk cc0 (first-matmul critical) issues first there;
            # the delay to gpsimd's wq cc0 is absorbed by the kt chain
            nc.gpsimd.dma_start(out=mask_sb[:], in_=mask_ext[:])
            # per-partition bias for the DVE exp groups: mask*128*log2e + B16
            nc.vector.tensor_scalar(
                out=m16[:],
                in0=mask_sb[:],
                scalar1=128.0 * LOG2E,
                scalar2=B16,
                op0=mybir.AluOpType.mult,
                op1=mybir.AluOpType.add,
            )
            # first chains (kt/qt pair0 sch0) need only wk+wq and ht q-chunk
            # 0: load that critical set first, striped over THREE queues
            # (scalar is idle during the load phase; 3 queues ~ HBM cap),
            # cc-interleaved so the woven chain matmuls start as chunks land
            q3 = [nc.sync, nc.gpsimd, nc.scalar]
            for cc in range(CCH):
                _w("wk", wk_ext, cc, q3[cc % 3])
                _w("wq", wq_ext, cc, q3[(cc + 1) % 3])
                _hq(cc, 0, q3[(cc + 2) % 3])
            # dummy 1-elem exp after scalar's critical-DMA share: pulls the
            # ~2.7us ACT table load into the load phase, before the first
            # real score activation
            warm = const.tile([128, 1], F32, tag="warm")
            nc.scalar.activation(
                out=warm[:],
                in_=mask_sb[:, 0:1],
                func=mybir.ActivationFunctionType.Exp,
            )
            # non-critical remainder on sync/gpsimd only (scalar must stay
            # clear for the exp stream). Urgency order: the kt sch1/2/3
            # chains woven INSIDE unit (0,0) consume kt columns for kc
            # 4..15, so ALL of ht q1/q2/q3 is needed by ~17-23us — earlier
            # than wv (v_gen drains at the END of unit (0,1), ~33us).
            # Stripe ht across both queues first; wv strictly behind it.
            # (Measured: wv racing ht q2/q3 cost ~5us of early PE stalls.)
            for q in range(1, QCH):
                for cc in range(CCH):
                    _hq(cc, q, nc.sync if (cc + q) % 2 == 0 else nc.gpsimd)
            for cc in range(CCH):
                _w("wv", wv_ext, cc, nc.sync if cc % 2 == 0 else nc.gpsimd)

            # v_aug[p, kc, h, 0:64] = v, [..., 64] = 1.0 (denominator column)
            vaug = const.tile([128, KCH, HEADS_PER_CORE, HD + 1], BF16, tag="vaug")
            nc.vector.memset(vaug[:, :, :, HD : HD + 1], 1.0)

            qt_sb = [
                qk.tile([128, S], BF16, name=f"qt{p}", tag=f"qt{p}")
                for p in range(N_PAIR)
            ]
            kt_sb = [
                qk.tile([128, S], BF16, name=f"kt{p}", tag=f"kt{p}")
                for p in range(N_PAIR)
            ]

            def chain_gen(p, name, sch):
                # lazy emission of one qT/kT projection chain: one
                # instruction per next() so it can be woven between score
                # pairs (the PE queue is strict FIFO)
                w = w_sb[name]
                dst = (qt_sb if name == "wq" else kt_sb)[p]
                ps = pj_ps.tile([128, QC], F32, tag="pj", name=f"pj_{name}{p}_{sch}")
                for cc in range(CCH):
                    nc.tensor.matmul(
                        ps[:],
                        w[:, cc, p * 128 : (p + 1) * 128],
                        ht_sb[:, cc, sch * QC : (sch + 1) * QC],
                        start=(cc == 0),
                        stop=(cc == CCH - 1),
                    )
                    yield
                nc.vector.tensor_copy(dst[:, sch * QC : (sch + 1) * QC], ps[:])
                yield

            def v_gen(kb):
                # one k-block of the v projection (all 6 heads)
                wv = w_sb["wv"]
                ps = pj_ps.tile(
                    [128, HEADS_PER_CORE * HD], F32, tag="pj", name=f"pv_{kb}"
                )
                for cc in range(CCH):
                    nc.tensor.matmul(
                        ps[:],
                        ht_sb[:, cc, kb * 128 : (kb + 1) * 128],
                        wv[:, cc, :],
                        start=(cc == 0),
                        stop=(cc == CCH - 1),
                    )
                    yield
                nc.vector.tensor_copy(
                    vaug[:, kb, :, 0:HD],
                    ps[:].rearrange("p (h d) -> p h d", h=HEADS_PER_CORE),
                )
                yield

            def weave(gens, budget):
                n = 0
                while gens and n < budget:
                    try:
                        next(gens[0])
                        n += 1
                    except StopIteration:
                        gens.pop(0)

            def drain(gens):
                while gens:
                    try:
                        next(gens[0])
                    except StopIteration:
                        gens.pop(0)

            def alloc_ex(p, j):
                return expp.tile([128, KCH, 2, QC], BF16, tag="ex", name=f"ex{p}_{j}")

            def ex_sl(ex, kc):
                return ex[:, kc]

            def scores_exp_part(p, j, ex, kcs):
                for kc in kcs:
                    sc = sc_ps.tile([128, 2, QC], F32, tag="sc", name=f"sc{p}_{j}_{kc}")
                    for h01 in range(2):
                        lo, hi = h01 * 64, h01 * 64 + 64
                        nc.tensor.matmul(
                            sc[:, h01, :],
                            kt_sb[p][lo:hi, kc * 128 : (kc + 1) * 128],
                            qt_sb[p][lo:hi, j * QC : (j + 1) * QC],
                            start=True,
                            stop=True,
                        )
                    if kc in DVE_KC:
                        nc.vector.tensor_scalar(
                            out=ex_sl(ex, kc).bitcast(I16),
                            in0=sc[:],
                            scalar1=A16,
                            scalar2=m16[:, kc : kc + 1],
                            op0=mybir.AluOpType.mult,
                            op1=mybir.AluOpType.add,
                        )
                    else:
                        nc.scalar.activation(
                            out=ex_sl(ex, kc),
                            in_=sc[:],
                            func=mybir.ActivationFunctionType.Exp,
                            scale=SCALE,
                            bias=mask_sb[:, kc : kc + 1],
                        )

            def scores_exp(p, j):
                ex = alloc_ex(p, j)
                scores_exp_part(p, j, ex, range(KCH))
                return ex

            def ctx_evac(cp, cj, cx0, cx1, on_vector=False):
                # final unit: DMAs ride sync+scalar so the gpsimd queue's
                # last DMA lands a unit earlier and its slow SWDGE teardown
                # drain (~3.6us) overlaps the tail instead of trailing it
                eng1 = nc.scalar if on_vector else nc.gpsimd
                for h01, cx, eng in ((0, cx0, nc.sync), (1, cx1, eng1)):
                    o_sb = outp.tile(
                        [HD + 1, QC], F32, tag="o", name=f"o{cp}_{cj}_{h01}"
                    )
                    # scalar engine (has slack) evacuates ctx psum so the
                    # vector queue stays clear for exp groups + proj casts;
                    # the final self-ctx unit uses the (by then idle)
                    # vector queue so the two tail evacs run in parallel
                    if on_vector:
                        nc.vector.tensor_copy(o_sb[:], cx[:])
                    else:
                        nc.scalar.copy(o_sb[:], cx[:])
                    eng.dma_start(
                        out=out_ext[2 * cp + h01, :, cj * QC : (cj + 1) * QC],
                        in_=o_sb[:],
                    )

            def fused_unit(p, j, cp, cj, cex, gens=(), budget=4, ctx2=None,
                           ctx_self=None):
                # scores+exp of unit (p, j) interleaved per-kc with the ctx
                # accumulation of unit (cp, cj) and woven projection chain
                # steps: the PE engine queue is strict FIFO, so emission
                # order must match the steady-state demand (one score pair
                # per ACT period, ctx + proj fill the rest) or ACT starves.
                gens = list(gens)
                ex = alloc_ex(p, j)
                cx0 = cx1 = None
                if cp is not None:
                    cx0 = cx_ps.tile([HD + 1, QC], F32, tag="cx0", name=f"cx0_{cp}_{cj}")
                    cx1 = cx_ps.tile([HD + 1, QC], F32, tag="cx1", name=f"cx1_{cp}_{cj}")
                dx0 = dx1 = None
                if ctx2 is not None:
                    dp, dj, _ = ctx2
                    dx0 = pj_ps.tile([HD + 1, QC], F32, tag="pj", name=f"cx0_{dp}_{dj}")
                    dx1 = pj_ps.tile([HD + 1, QC], F32, tag="pj", name=f"cx1_{dp}_{dj}")
                if ctx_self:
                    # this unit's own ctx rides the pj banks, consuming its
                    # own ex with a 2-kc lag (the exp of kc is ready by the
                    # time the PE reaches kc+2) — kills the serial PE tail
                    dx0 = pj_ps.tile([HD + 1, QC], F32, tag="pj", name=f"cxs0_{p}_{j}")
                    dx1 = pj_ps.tile([HD + 1, QC], F32, tag="pj", name=f"cxs1_{p}_{j}")

                def ctx_self_mm(kcc):
                    for h01, cx in ((0, dx0), (1, dx1)):
                        nc.tensor.matmul(
                            cx[:],
                            vaug[:, kcc, 2 * p + h01, :],
                            ex_sl(ex, kcc)[:, h01, :],
                            start=(kcc == 0),
                            stop=(kcc == KCH - 1),
                        )

                # kc-PAIR clustering: both score pairs (64-row-tiled mode)
                # back to back, then all 128-row work (proj weave + ctx) for
                # the pair — 2 PE mode-switch drains per 2 kcs instead of 4
                for kcp in range(KCH // 2):
                    kc0, kc1 = 2 * kcp, 2 * kcp + 1
                    scores_exp_part(p, j, ex, [kc0])
                    scores_exp_part(p, j, ex, [kc1])
                    weave(gens, budget)
                    for kc in (kc0, kc1):
                        if cp is not None:
                            for h01, cx in ((0, cx0), (1, cx1)):
                                nc.tensor.matmul(
                                    cx[:],
                                    vaug[:, kc, 2 * cp + h01, :],
                                    ex_sl(cex, kc)[:, h01, :],
                                    start=(kc == 0),
                                    stop=(kc == KCH - 1),
                                )
                        if ctx2 is not None:
                            dp, dj, dex = ctx2
                            for h01, cx in ((0, dx0), (1, dx1)):
                                nc.tensor.matmul(
                                    cx[:],
                                    vaug[:, kc, 2 * dp + h01, :],
                                    ex_sl(dex, kc)[:, h01, :],
                                    start=(kc == 0),
                                    stop=(kc == KCH - 1),
                                )
                    if ctx_self and kcp >= 1:
                        ctx_self_mm(kc0 - 2)
                        ctx_self_mm(kc1 - 2)
                drain(gens)
                if ctx_self:
                    ctx_self_mm(KCH - 2)
                    ctx_self_mm(KCH - 1)
                if cp is not None:
                    ctx_evac(cp, cj, cx0, cx1)
                if ctx2 is not None:
                    dp, dj, _ = ctx2
                    ctx_evac(dp, dj, dx0, dx1)
                if ctx_self:
                    ctx_evac(p, j, dx0, dx1, on_vector=True)
                return ex

            # Pair 0 is special-ordered so ACT starts as early as possible:
            # scores/exp need only qT/kT; v-projection matmuls fill PE gaps
            # while ACT chews exps; ctx comes after proj_v (vaug dependency).
            # Next pair's projection chains are spread through the current
            # pair's attention units so the scheduler can hide them in the
            # ACT-gated gaps instead of paying for them at pair boundaries.
            # Startup: first kT/qT chains run as blocks (critical path),
            # then everything — remaining chains, the v projection, next
            # pair's chains — is woven between score pairs at kc grain.
            drain([chain_gen(0, "wk", 0)])
            drain([chain_gen(0, "wq", 0)])
            ex00 = alloc_ex(0, 0)
            g = [chain_gen(0, "wk", 1), chain_gen(0, "wk", 2),
                 chain_gen(0, "wk", 3), chain_gen(0, "wq", 1)]
            for kcp in range(KCH // 2):
                scores_exp_part(0, 0, ex00, [2 * kcp])
                scores_exp_part(0, 0, ex00, [2 * kcp + 1])
                weave(g, 4)
            drain(g)
            ex01 = alloc_ex(0, 1)
            g = [chain_gen(0, "wq", 2)] + [v_gen(kb) for kb in range(KCH)]
            for kcp in range(KCH // 2):
                scores_exp_part(0, 1, ex01, [2 * kcp])
                scores_exp_part(0, 1, ex01, [2 * kcp + 1])
                weave(g, 6)
            drain(g)  # ctx(0,0) in the next unit needs all of v
            ex02 = fused_unit(0, 2, 0, 0, ex00, [chain_gen(0, "wq", 3)])
            ex03 = fused_unit(
                0, 3, 0, 1, ex01,
                [chain_gen(1, "wk", 0), chain_gen(1, "wk", 1), chain_gen(1, "wq", 0)],
            )
            ex10 = fused_unit(
                1, 0, 0, 2, ex02,
                [chain_gen(1, "wk", 2), chain_gen(1, "wk", 3), chain_gen(1, "wq", 1)],
            )
            ex11 = fused_unit(
                1, 1, 0, 3, ex03, [chain_gen(1, "wq", 2), chain_gen(2, "wk", 0)]
            )
            ex12 = fused_unit(
                1, 2, 1, 0, ex10, [chain_gen(1, "wq", 3), chain_gen(2, "wk", 1)]
            )
            ex13 = fused_unit(
                1, 3, 1, 1, ex11, [chain_gen(2, "wk", 2), chain_gen(2, "wq", 0)]
            )
            ex20 = fused_unit(
                2, 0, 1, 2, ex12, [chain_gen(2, "wk", 3), chain_gen(2, "wq", 1)]
            )
            ex21 = fused_unit(
                2, 1, 1, 3, ex13, [chain_gen(2, "wq", 2), chain_gen(2, "wq", 3)]
            )
            # tail shrink: the last two units each carry TWO ctx units —
            # one on the cx banks, one on the now-idle proj banks; the very
            # last unit self-consumes its own ex at a 2-kc lag so only two
            # ctx chunks trail the final scores
            ex22 = fused_unit(2, 2, 2, 0, ex20, ctx2=(2, 1, ex21))
            ex23 = fused_unit(2, 3, 2, 2, ex22, ctx_self=True)

    nc.compile()
    return nc


def _get_nc():
    if "nc" not in _NC_CACHE:
        _NC_CACHE["nc"] = _build_nc()
    return _NC_CACHE["nc"]


def _make_in_maps(hidden, mask, Wq, Wk, Wv):
    bf16 = ml_dtypes.bfloat16
    in_maps = []
    for c in range(N_CORES):
        b, hg = c // 2, c % 2
        cols = slice(hg * HEADS_PER_CORE * HD, (hg + 1) * HEADS_PER_CORE * HD)
        mc = np.ascontiguousarray(
            mask[b, 0, 0].astype(np.float32).reshape(KCH, 128).T
        )
        in_maps.append(
            {
                "ht": np.ascontiguousarray(hidden[b].T).astype(bf16),
                "wq": np.ascontiguousarray(Wq[:, cols]).astype(bf16),
                "wk": np.ascontiguousarray(Wk[:, cols]).astype(bf16),
                "wv": np.ascontiguousarray(Wv[:, cols]).astype(bf16),
                "mask": mc,
            }
        )
    return in_maps


def _gather(results):
    out = np.empty((B, S, H), dtype=np.float32)
    for c in range(N_CORES):
        b, hg = c // 2, c % 2
        r = results[c]["out"]  # [6, 65, S]
        num = r[:, :HD, :]  # [6, 64, S]
        den = r[:, HD : HD + 1, :]  # [6, 1, S]
        ctx = np.transpose(num / den, (2, 0, 1)).reshape(S, HEADS_PER_CORE * HD)
        out[b, :, hg * HEADS_PER_CORE * HD : (hg + 1) * HEADS_PER_CORE * HD] = ctx
    return out


def _run_device(hidden, mask, Wq, Wk, Wv, trace=False):
    nc = _get_nc()
    in_maps = _make_in_maps(hidden, mask, Wq, Wk, Wv)
    res = run_bass_kernel_spmd(nc, in_maps, core_ids=list(range(N_CORES)), trace=trace)
    return _gather(res.results), res


def _numpy_fallback(hidden_states, attention_mask, Wq, bq, Wk, bk, Wv, bv):
    def split_heads(x):
        return x.reshape(B, S, NH, HD).transpose(0, 2, 1, 3)

    q = split_heads(hidden_states @ Wq + bq)
    k = split_heads(hidden_states @ Wk + bk)
    v = split_heads(hidden_states @ Wv + bv)
    scores = np.einsum("bhqd,bhkd->bhqk", q, k) / np.sqrt(HD) + attention_mask
    scores -= scores.max(axis=-1, keepdims=True)
    e = np.exp(scores)
    probs = e / e.sum(axis=-1, keepdims=True)
    ctx = np.einsum("bhqk,bhkd->bhqd", probs, v)
    return ctx.transpose(0, 2, 1, 3).reshape(B, S, H).astype(np.float32)


def kernel(hidden_states, attention_mask, Wq, bq, Wk, bk, Wv, bv):
    hidden = np.asarray(hidden_states, dtype=np.float32)
    mask = np.asarray(attention_mask, dtype=np.float32)
    Wq = np.asarray(Wq, dtype=np.float32)
    Wk = np.asarray(Wk, dtype=np.float32)
    Wv = np.asarray(Wv, dtype=np.float32)
    bq, bk, bv = (np.asarray(x, dtype=np.float32) for x in (bq, bk, bv))
    if np.any(bq) or np.any(bk) or np.any(bv):
        # projection biases are zero for this problem; keep a correct
        # fallback rather than a dead device path
        return _numpy_fallback(hidden, mask, Wq, bq, Wk, bk, Wv, bv)
    out, _ = _run_device(hidden, mask, Wq, Wk, Wv)
    return out

